# revision 9
# baseline (speedup 1.0000x reference)
"""Trainium2 Bass kernel for a 3-layer binarized CNN.

Network (reference):
    x  : [32, 3, 512, 512] fp32
    l1 : clip(conv(x, sign(w1)))            -> [32,16,510,510]
    l2 : clip(conv(sign(l1), sign(w2)))     -> [32,23,508,508]
    l3 : clip(conv(sign(l2), sign(w3)))     -> [32,2,506,506]
    out: l3.reshape(32, -1)

Strategy (pure data parallel, 4 images per NeuronCore x 8 cores):
  * All convs are Toeplitz-stationary matmuls on the tensor engine.
    The contraction (partition) axis holds a window of image ROWS (the
    original H axis) interleaved with input channels; the moving axis
    streams image columns (original W).  The 3 kernel taps along W are
    handled by 3 PSUM-accumulated matmuls whose rhs is shifted by
    dx in the free dimension.
  * Layer 1 is numerically sensitive (only the SIGN of its output
    matters downstream, so it must be ~fp32-accurate).  The fp32 input
    is split x = hi + lo into two fp16 terms which are folded into the
    contraction axis (weights are +-1, exactly representable), giving
    fp32-class accuracy at fp16 matmul speed.
  * Layers 2/3 have +-1 x +-1 products with small-integer sums; fp8
    inputs with fp32 PSUM accumulation are bit-exact.  sign()/clip()
    of an integer-valued tensor are both exactly clip(x,-1,1).
  * Intermediates bounce through DRAM in a [row, channel, col] layout
    so every DMA moves long contiguous runs.
"""

import numpy as np
import ml_dtypes

import concourse.bacc as bacc
import concourse.mybir as mybir
import concourse.tile as tile
from concourse.bass_utils import run_bass_kernel_spmd

F32 = mybir.dt.float32
F16 = mybir.dt.float16
F8 = mybir.dt.float8e4
NP_F8 = ml_dtypes.float8_e4m3

N_CORES = 8
IMG_PER_CORE = 4

AL1, AO1 = 10, 8     # L1: K rows-window, M rows-out per block
AL2, AO2 = 7, 5      # L2
AL3, AO3 = 32, 30    # L3
C1, C2, C3 = 3, 16, 23
O1, O2, O3 = 16, 23, 2


def _toeplitz_weights(w1, w2, w3):
    """Build the stationary Toeplitz matrices (host side)."""
    s1 = np.sign(w1).astype(np.float32)  # [16,3,3,3]
    s2 = np.sign(w2).astype(np.float32)  # [23,16,3,3]
    s3 = np.sign(w3).astype(np.float32)  # [2,23,3,3]

    # T1[(term*32 + al*3 + c), dx, (aol*16 + o)]  (rows 30,31,62,63,94,95 zero)
    t1 = np.zeros((96, 3, AO1 * O1), np.float32)
    for al in range(AL1):
        for aol in range(AO1):
            dy = al - aol
            if 0 <= dy <= 2:
                for c in range(C1):
                    for t in range(3):
                        for o in range(O1):
                            t1[t * 32 + al * 3 + c, :, aol * 16 + o] = s1[o, c, dy, :]
    # T2[(al*16 + c), dx, (aol*23 + o)]
    t2 = np.zeros((AL2 * C2, 3, AO2 * O2), np.float32)
    for al in range(AL2):
        for aol in range(AO2):
            dy = al - aol
            if 0 <= dy <= 2:
                for c in range(C2):
                    for o in range(O2):
                        t2[al * 16 + c, :, aol * 23 + o] = s2[o, c, dy, :]
    # T3[(al*4 + cl), (cc*3 + dx), (aol*2 + o)]
    t3 = np.zeros((AL3 * 4, 6, 3, AO3 * O3), np.float32)
    for al in range(AL3):
        for aol in range(AO3):
            dy = al - aol
            if 0 <= dy <= 2:
                for cc in range(6):
                    for cl in range(4):
                        c = cc * 4 + cl
                        if c < C3:
                            for o in range(O3):
                                t3[al * 4 + cl, cc, :, aol * 2 + o] = s3[o, c, dy, :]
    return (
        t1.reshape(96, 3 * 128).astype(np.float16),
        t2.reshape(112, 3 * 115).astype(NP_F8),
        t3.reshape(128, 18 * 60).astype(NP_F8),
    )


def _build_program(n_img, A, B):
    """Emit the per-core SPMD Bass program (all 3 layers, n_img images)."""
    nblk1 = -(-(A - 2) // AO1)          # L1 row-blocks (of 8), padded
    nch1 = -(-nblk1 // 2)               # L1 chunks = 2 blocks each
    a_pad = 16 * nch1 + 2               # input rows needed (zero padded)
    a1 = AO1 * nblk1                    # s1 row count (incl. garbage tail)
    nblk2 = -(-(A - 4) // AO2)
    nblk3 = -(-(A - 6) // AO3)
    s2a = max(AO2 * nblk2, AO3 * (nblk3 - 1) + AL3)  # s2 rows incl. zero pad
    n1, n2, n3 = B - 2, B - 4, B - 6

    assert AL2 + AO2 * (nblk2 - 1) <= a1, "L2 reads past s1"

    nc = bacc.Bacc("TRN2", target_bir_lowering=False, debug=False)

    xt = nc.dram_tensor("xt", [n_img, 3, a_pad, B], F32, kind="ExternalInput")
    t1w = nc.dram_tensor("t1w", [96, 3 * 128], F16, kind="ExternalInput")
    t2w = nc.dram_tensor("t2w", [112, 3 * 115], F8, kind="ExternalInput")
    t3w = nc.dram_tensor("t3w", [128, 18 * 60], F8, kind="ExternalInput")
    outp = nc.dram_tensor(
        "outp", [n_img, AO3 * nblk3, 2, n3], F32, kind="ExternalOutput"
    )
    s1d = [
        nc.dram_tensor(f"s1_{i}", [a1, 16, n1], F8, kind="Internal")
        for i in range(n_img)
    ]
    s2d = [
        nc.dram_tensor(f"s2_{i}", [s2a, 24, n2], F8, kind="Internal")
        for i in range(n_img)
    ]

    with tile.TileContext(nc) as tc:
        with (
            tc.tile_pool(name="const", bufs=1) as cpool,
            tc.tile_pool(name="l1x", bufs=3) as p1x,
            tc.tile_pool(name="l1s", bufs=4) as p1s,
            tc.tile_pool(name="l2", bufs=4) as p2,
            tc.tile_pool(name="l3", bufs=14) as p3,
            tc.tile_pool(name="ps1", bufs=3, space="PSUM") as ps1p,
            tc.tile_pool(name="ps2", bufs=2, space="PSUM") as ps2p,
            tc.tile_pool(name="ps3", bufs=2, space="PSUM") as ps3p,
        ):
            t1sb = cpool.tile([96, 3 * 128], F16)
            t2sb = cpool.tile([112, 3 * 115], F8)
            t3sb = cpool.tile([128, 18 * 60], F8)
            ztile = cpool.tile([128, B], F8)
            nc.sync.dma_start(t1sb[:], t1w.ap()[:])
            nc.sync.dma_start(t2sb[:], t2w.ap()[:])
            nc.sync.dma_start(t3sb[:], t3w.ap()[:])
            nc.vector.memset(ztile[:], 0.0)
            # persistent L1 rhs ring: [term(3) x 32, B] fp16, hi at rows
            # 0..29, mid at 32..61, lo at 64..93; spacer rows stay zero.
            NRHS = 3
            rhs_ring = []
            for ri in range(NRHS):
                rt = cpool.tile([96, B], F16, name=f"rhs1_{ri}")
                nc.vector.memset(rt[:], 0.0)
                rhs_ring.append(rt)

            for img in range(n_img):
                s1, s2 = s1d[img].ap(), s2d[img].ap()
                # ---- zero pads of s2: channel-23 plane + tail rows ----
                for r in range(0, s2a, 128):
                    cnt = min(128, s2a - r)
                    nc.sync.dma_start(s2[r : r + cnt, 23, :], ztile[:cnt, :n2])
                for a in range(AO2 * nblk2, s2a):
                    nc.sync.dma_start(s2[a, :, :], ztile[:24, :n2])

                # ---------------- layer 1 ----------------
                for blk in range(nblk1):
                    a0 = 8 * blk
                    x32 = p1x.tile([30, B], F32, tag="x32")
                    nc.sync.dma_start(
                        x32[:],
                        xt.ap()[img, :, a0 : a0 + 10, :].transpose([1, 0, 2]),
                    )
                    rhs16 = rhs_ring[blk % NRHS]
                    hi16 = p1x.tile([30, B], F16, tag="hi16")
                    nc.vector.tensor_copy(hi16[:], x32[:])
                    nc.vector.tensor_copy(rhs16[0:30, :], hi16[:])
                    d32 = p1x.tile([30, B], F32, tag="d32")
                    nc.vector.scalar_tensor_tensor(
                        d32[:], x32[:], 1.0, hi16[:],
                        op0=mybir.AluOpType.mult,
                        op1=mybir.AluOpType.subtract,
                    )
                    mid16 = p1x.tile([30, B], F16, tag="mid16")
                    nc.vector.tensor_copy(mid16[:], d32[:])
                    nc.vector.tensor_copy(rhs16[32:62, :], mid16[:])
                    nc.vector.scalar_tensor_tensor(
                        rhs16[64:94, :], d32[:], 1.0, mid16[:],
                        op0=mybir.AluOpType.mult,
                        op1=mybir.AluOpType.subtract,
                    )
                    ps = ps1p.tile([128, n1], F32, tag="ps1")
                    for dx in range(3):
                        nc.tensor.matmul(
                            ps[:],
                            t1sb[:, 128 * dx : 128 * dx + 128],
                            rhs16[:, dx : dx + n1],
                            start=(dx == 0),
                            stop=(dx == 2),
                        )
                    pos16 = p1s.tile([128, n1], F16, tag="pos16")
                    nc.vector.tensor_scalar(
                        pos16[:], ps[:], 0.0, None, op0=mybir.AluOpType.is_gt
                    )
                    sg8 = p1s.tile([128, n1], F8, tag="sg8")
                    nc.vector.tensor_scalar(
                        sg8[:], pos16[:], 2.0, -1.0,
                        op0=mybir.AluOpType.mult,
                        op1=mybir.AluOpType.add,
                    )
                    nc.sync.dma_start(s1[a0 : a0 + 8, :, :], sg8[:])

                # ---------------- layer 2 ----------------
                for b in range(nblk2):
                    rhs8 = p2.tile([112, n1], F8, tag="rhs8")
                    nc.sync.dma_start(rhs8[:], s1[5 * b : 5 * b + 7, :, :])
                    ps = ps2p.tile([115, n2], F32, tag="ps2")
                    for dx in range(3):
                        nc.tensor.matmul(
                            ps[:],
                            t2sb[:, 115 * dx : 115 * dx + 115],
                            rhs8[:, dx : dx + n2],
                            start=(dx == 0),
                            stop=(dx == 2),
                        )
                    sg2 = p2.tile([115, n2], F8, tag="sg2")
                    nc.vector.tensor_scalar(
                        sg2[:], ps[:], -1.0, 1.0,
                        op0=mybir.AluOpType.max,
                        op1=mybir.AluOpType.min,
                    )
                    nc.sync.dma_start(s2[5 * b : 5 * b + 5, 0:23, :], sg2[:])

                # ---------------- layer 3 ----------------
                for bb in range(nblk3):
                    rts = []
                    for cc in range(6):
                        rt = p3.tile([128, n2], F8, tag="rhs3")
                        nc.sync.dma_start(
                            rt[:], s2[30 * bb : 30 * bb + 32, 4 * cc : 4 * cc + 4, :]
                        )
                        rts.append(rt)
                    ps = ps3p.tile([60, n3], F32, tag="ps3")
                    for cc in range(6):
                        for dx in range(3):
                            nc.tensor.matmul(
                                ps[:],
                                t3sb[:, 60 * (cc * 3 + dx) : 60 * (cc * 3 + dx) + 60],
                                rts[cc][:, dx : dx + n3],
                                start=(cc == 0 and dx == 0),
                                stop=(cc == 5 and dx == 2),
                            )
                    oc = p3.tile([60, n3], F32, tag="oc")
                    nc.vector.tensor_scalar(
                        oc[:], ps[:], -1.0, 1.0,
                        op0=mybir.AluOpType.max,
                        op1=mybir.AluOpType.min,
                    )
                    nc.sync.dma_start(
                        outp.ap()[img, 30 * bb : 30 * bb + 30, :, :], oc[:]
                    )

    nc.compile()
    return nc


_CACHE = {}


def _get_program(n_img, A, B):
    key = (n_img, A, B)
    if key not in _CACHE:
        _CACHE[key] = _build_program(n_img, A, B)
    return _CACHE[key]


def make_in_maps(x, w1, w2, w3, n_cores=N_CORES, a_pad=None):
    """x: [N,3,A,B] fp32 -> list of per-core input maps."""
    n, _, A, B = x.shape
    per = n // n_cores
    nblk1 = -(-(A - 2) // AO1)
    nch1 = -(-nblk1 // 2)
    if a_pad is None:
        a_pad = 16 * nch1 + 2
    t1, t2, t3 = _toeplitz_weights(
        np.asarray(w1, np.float32), np.asarray(w2, np.float32), np.asarray(w3, np.float32)
    )
    xp = np.zeros((n, 3, a_pad, B), np.float32)
    xp[:, :, :A, :] = np.asarray(x, np.float32)
    maps = []
    for i in range(n_cores):
        maps.append(
            {
                "xt": np.ascontiguousarray(xp[per * i : per * (i + 1)]),
                "t1w": t1,
                "t2w": t2,
                "t3w": t3,
            }
        )
    return maps


last_results = None


def kernel(inputs, w1, w2, w3):
    global last_results
    x = np.asarray(inputs, np.float32)
    n, _, A, B = x.shape
    per = n // N_CORES
    nc = _get_program(per, A, B)
    maps = make_in_maps(x, w1, w2, w3)
    res = run_bass_kernel_spmd(nc, maps, core_ids=list(range(N_CORES)))
    last_results = res
    a3, b3 = A - 6, B - 6
    out = np.empty((n, 2, a3, b3), np.float32)
    for i, r in enumerate(res.results):
        o = r["outp"][:, :a3, :, :]          # [per, a3, 2, b3]
        out[per * i : per * (i + 1)] = o.transpose(0, 2, 1, 3)
    return out.reshape(n, -1)
